# revision 14
# baseline (speedup 1.0000x reference)
"""Trainium2 Bass kernel for nn_Delta: y = x @ (base + (U*S) @ V^T)^T.

Shapes (hardcoded): x [2,256,8192] f32, base [8192,8192] f32,
all_U [8192,1024] f32, all_S [1024] f32, all_V [8192,1024] f32.
Output: [2,256,8192] f32.

Strategy (8 NeuronCores, tensor-parallel over OUT):
  Never materialize w.  Factor:  y = x @ base^T + ((x @ V) * S) @ U^T.
  - OUT is sharded 8 ways (1024 cols per core) for base / U.
  - t = x @ V is sharded over RANK: core k computes t[:, k*128:(k+1)*128]
    (reading only its 128-column slice of V), then an on-chip AllGather
    makes the full t [512, 1024] available to every core.
  - Each core then accumulates, in PSUM: y_k = x @ baseT_k  (64 K-tiles)
    followed by  t @ uT_k  (8 rank tiles) and writes its [512, 1024] slice.
  All matmul operands are pre-packed bf16 host-side (halves HBM traffic and
  runs the PE at full rate); PSUM accumulation is fp32; output is fp32.
"""

import os

import ml_dtypes
import numpy as np

P = 128
OUT, IN, RANK = 8192, 8192, 1024
B, S = 2, 256
T = B * S  # 512 tokens
NCORES = 8
O_SH = OUT // NCORES  # 1024 out cols per core
NI = IN // P  # 64 contraction tiles
NT = T // P  # 4 token tiles
NO = O_SH // 512  # 2 out half-tiles per core
NR = RANK // P  # 8 rank tiles

_CACHE: dict = {}


def _build_nc(repeat=1, collective=True):
    """Build the Bass program.  repeat>1 unrolls the whole compute N times in
    one NEFF (same inputs/outputs) — used only to measure steady-state
    per-iteration device time above the ~90ms axon launch overhead.
    collective=False replaces the AllGather with local DMAs (wrong numerics,
    same traffic shape) so the single-core cost-model simulator can run."""
    import concourse.mybir as mybir
    import concourse.tile as tile
    from concourse import bacc

    dt = mybir.dt
    BF = dt.bfloat16
    F32 = dt.float32

    nc = bacc.Bacc(
        "TRN2", target_bir_lowering=False, debug=False, num_devices=NCORES
    )

    # Host-packed per-core inputs.  Layouts put the matmul contraction dim on
    # SBUF partitions so every DMA is a plain 2D strided copy:
    #   xt[p, i*512 + t] = x[t, i*128 + p]           (lhsT tiles for all MMs)
    #   vk[p, i*128 + r] = V[i*128 + p, k*128 + r]   (stationary V slice)
    #   bt[p, i*1024 + o] = base[k*1024 + o, i*128 + p]  (moving baseT tiles)
    #   ut[p, j*1024 + o] = (U*S)[k*1024 + o, j*128 + p]
    xt = nc.dram_tensor("xt", [P, NI * T], BF, kind="ExternalInput")
    vk = nc.dram_tensor("vk", [P, NI * P], BF, kind="ExternalInput")
    bt = nc.dram_tensor("bt", [P, NI * O_SH], BF, kind="ExternalInput")
    ut = nc.dram_tensor("ut", [P, NR * O_SH], BF, kind="ExternalInput")
    y = nc.dram_tensor("y", [T, O_SH], F32, kind="ExternalOutput")

    with tile.TileContext(nc) as tc:
        with (
            tc.tile_pool(name="resident", bufs=1) as res_pool,
            tc.tile_pool(name="bt_pool", bufs=36) as bt_pool,
            tc.tile_pool(name="y_pool", bufs=4) as y_pool,
            tc.tile_pool(name="psum", bufs=1, space="PSUM") as ps_pool,
            tc.tile_pool(name="dram", bufs=2, space="DRAM") as dram_pool,
        ):
            # --- resident SBUF loads (once per launch) ---
            # Interleave vk/xt group loads so the t-phase's first matmuls can
            # start after ~1.3MB instead of after the full 12.6MB; ut (only
            # needed ~100us in, at lora time) loads last.
            xt_sb, vk_sb = [], []
            for g in range(8):  # 8 groups x 8 i-tiles
                vk_g = res_pool.tile([P, 8 * P], BF, name=f"vk{g}")
                nc.sync.dma_start(
                    out=vk_g[:], in_=vk[:, g * 8 * P : (g + 1) * 8 * P]
                )
                vk_sb.append(vk_g)
                xt_g = res_pool.tile([P, 8 * T], BF, name=f"xt{g}")
                nc.sync.dma_start(out=xt_g[:], in_=xt[:, g * 8 * T : (g + 1) * 8 * T])
                xt_sb.append(xt_g)

            def xt_slice(i, lo, width):
                g, j = divmod(i, 8)
                return xt_sb[g][:, j * T + lo : j * T + lo + width]

            def vk_slice(i):
                g, j = divmod(i, 8)
                return vk_sb[g][:, j * P : (j + 1) * P]

            ut_sb = res_pool.tile([P, NR * O_SH], BF, name="ut_sb")
            nc.sync.dma_start(out=ut_sb[:], in_=ut[:])

            for it in range(repeat):
                # t-phase (tT_local[r, tok] = sum_i V[i, r_k] x[tok, i]) is
                # interleaved into the first half of the base loop, 2 of its 64
                # K-tiles per base K-tile, so its matmuls fill what would
                # otherwise be DMA-starved PE time at kernel start and the
                # AllGather still launches at ~55% of the base loop.  Its PSUM
                # bank is freed at the halfway point, so bank 7 (tt=3, ot=1)
                # defers its first-half base accumulation: in the second half
                # it runs 2 MMs per K-tile (current i + makeup i-32, addition
                # commutes), with bt tiles 0..31 held resident until consumed.
                t_ps = ps_pool.tile([P, T], F32, name=f"t_ps_{it}", tag="ps7")
                y_ps = [
                    ps_pool.tile([P, 512], F32, name=f"y_ps{b}_{it}", tag=f"ps{b}")
                    for b in range(8)
                ]
                bt_hold = {}
                for i in range(NI):
                    bt_t = bt_pool.tile([P, O_SH], BF, name="bt_t", tag="bt_t")
                    nc.sync.dma_start(
                        out=bt_t[:], in_=bt[:, i * O_SH : (i + 1) * O_SH]
                    )
                    if i < NI // 2:
                        bt_hold[i] = bt_t
                    for tt in range(NT):
                        lhsT = xt_slice(i, tt * P, P)
                        for ot in range(NO):
                            b = tt * NO + ot
                            if b == 7 and i < NI // 2:
                                continue  # deferred to second half
                            nc.tensor.matmul(
                                y_ps[b][:],
                                lhsT,
                                bt_t[:, ot * 512 : (ot + 1) * 512],
                                start=(i == 0 if b != 7 else i == NI // 2),
                                stop=False,
                            )
                    if i < NI // 2:
                        for s in (2 * i, 2 * i + 1):
                            nc.tensor.matmul(
                                t_ps[:],
                                vk_slice(s),
                                xt_slice(s, 0, T),
                                start=(s == 0),
                                stop=(s == NI - 1),
                            )
                        if i == NI // 2 - 1:
                            t_loc = res_pool.tile(
                                [P, T], BF, name=f"t_loc_{it}", tag="t_loc", bufs=2
                            )
                            nc.vector.tensor_copy(t_loc[:], t_ps[:])
                            t_in = dram_pool.tile(
                                [P, T], BF, name=f"t_in_{it}", tag="t_in"
                            )
                            t_all = dram_pool.tile(
                                [RANK, T], BF, name=f"t_all_{it}", tag="t_all",
                                addr_space="Shared" if collective else "Local",
                            )
                            nc.sync.dma_start(out=t_in[:], in_=t_loc[:])
                            if collective:
                                nc.gpsimd.collective_compute(
                                    "AllGather",
                                    mybir.AluOpType.bypass,
                                    replica_groups=[list(range(NCORES))],
                                    ins=[t_in.opt()],
                                    outs=[t_all.opt()],
                                )
                            else:
                                for j in range(NR):
                                    nc.sync.dma_start(
                                        out=t_all[j * P : (j + 1) * P, :],
                                        in_=t_in[:],
                                    )
                            t_all_sb = res_pool.tile(
                                [P, NR * T], BF, name=f"t_all_sb_{it}",
                                tag="t_all_sb", bufs=2,
                            )
                            nc.sync.dma_start(
                                out=t_all_sb[:].rearrange("p (n m) -> p n m", n=NR),
                                in_=t_all.rearrange("(n p) m -> p n m", p=P),
                            )
                    else:
                        j = i - NI // 2
                        nc.tensor.matmul(
                            y_ps[7][:],
                            xt_slice(j, 3 * P, P),
                            bt_hold[j][:, 512:1024],
                            start=False,
                            stop=False,
                        )
                # lora accumulation, bank-major so each bank finishes (and can
                # evict + DMA out) while later banks still accumulate
                for tt in range(NT):
                    for ot in range(NO):
                        for j in range(NR):
                            lhsT = t_all_sb[
                                :, j * T + tt * P : j * T + (tt + 1) * P
                            ]
                            nc.tensor.matmul(
                                y_ps[tt * NO + ot][:],
                                lhsT,
                                ut_sb[
                                    :,
                                    j * O_SH + ot * 512 : j * O_SH + (ot + 1) * 512,
                                ],
                                start=False,
                                stop=(j == NR - 1),
                            )
                        y_sb = y_pool.tile([P, 512], F32, name="y_sb", tag="y_sb")
                        nc.vector.tensor_copy(y_sb[:], y_ps[tt * NO + ot][:])
                        nc.sync.dma_start(
                            out=y[tt * P : (tt + 1) * P, ot * 512 : (ot + 1) * 512],
                            in_=y_sb[:],
                        )

    nc.compile()
    return nc


def _get_nc():
    if "nc" not in _CACHE:
        _CACHE["nc"] = _build_nc()
    return _CACHE["nc"]


def _pack_inputs(x, base, all_U, all_S, all_V):
    """Shard + pre-transpose + bf16-cast all inputs on the host."""
    bf16 = ml_dtypes.bfloat16
    x = np.ascontiguousarray(np.asarray(x, dtype=np.float32)).reshape(T, IN)
    base = np.asarray(base, dtype=np.float32)
    us = np.asarray(all_U, dtype=np.float32) * np.asarray(
        all_S, dtype=np.float32
    )[None, :]
    V = np.asarray(all_V, dtype=np.float32)

    xb = x.astype(bf16)
    baseb = base.astype(bf16)
    usb = us.astype(bf16)
    Vb = V.astype(bf16)

    xt = np.ascontiguousarray(
        xb.reshape(T, NI, P).transpose(2, 1, 0)
    ).reshape(P, NI * T)

    in_maps = []
    for k in range(NCORES):
        vk = np.ascontiguousarray(
            Vb[:, k * P : (k + 1) * P].reshape(NI, P, P).transpose(1, 0, 2)
        ).reshape(P, NI * P)
        bt = np.ascontiguousarray(
            baseb[k * O_SH : (k + 1) * O_SH, :]
            .reshape(O_SH, NI, P)
            .transpose(2, 1, 0)
        ).reshape(P, NI * O_SH)
        utk = np.ascontiguousarray(
            usb[k * O_SH : (k + 1) * O_SH, :]
            .reshape(O_SH, NR, P)
            .transpose(2, 1, 0)
        ).reshape(P, NR * O_SH)
        in_maps.append({"xt": xt, "vk": vk, "bt": bt, "ut": utk})
    return in_maps


def kernel(x, base, all_U, all_S, all_V):
    from concourse.bass_utils import run_bass_kernel_spmd

    nc = _get_nc()
    in_maps = _pack_inputs(x, base, all_U, all_S, all_V)
    res = run_bass_kernel_spmd(nc, in_maps, core_ids=list(range(NCORES)))
    _CACHE["last_results"] = res
    y = np.concatenate([res.results[k]["y"] for k in range(NCORES)], axis=1)
    return np.ascontiguousarray(y.reshape(B, S, OUT))
